# revision 1
# baseline (speedup 1.0000x reference)
"""Trainium2 Bass kernel for nn_JastrowFactorGraph (MGCN-style Jastrow factor).

Strategy (8 NeuronCores, pure data parallel over the 512-walker batch):
  - 64 walkers per core; SBUF layout uses 128 partitions = 64 features x 2
    walkers ("walker-set"); 32 sets per core processed as batched phases.
  - Dense symmetric filter grids: e-e is the complete graph on 30 electrons
    (30x30 grid, diag zeroed), e-n is bipartite (30x10 grid used in both
    directions). Gather/scatter become broadcast-AP multiplies + free-dim
    reductions on the vector engine.
  - RBF expansion exp(-(d-c_k)^2) is computed as exp(PE_matmul + bias):
    a K=4 matmul with rows {-dsq, 2c*d} per walker slot and the -c_k^2 term
    folded into the activation bias (per-partition vector).
  - filt = tanh(rbf @ wf + bf) and layer updates h += tanh(agg @ wl + bl)
    are block-diagonal PE matmuls contracting the feature dim on partitions.
  - Readout sum_n h[n] @ wr is a free-dim reduce + one K=128 matmul; the two
    graph scalars accumulate in PSUM and exp() is applied on-chip.
"""

import contextlib

import numpy as np

import concourse.bass as bass
import concourse.mybir as mybir
from concourse.bass_utils import run_bass_kernel_spmd

N_CORES = 8
NB = 512
NW = NB // N_CORES      # walkers per core = 64
NSETS = NW // 2         # 32 walker-sets (2 walkers per set)
NE = 30                 # electrons (ee nodes)
NA = 10                 # atoms
F = 64                  # features
K = 64                  # RBF size
CELLS_EE = NE * NE      # 900, j-major (j outer, i inner)
CELLS_EN = NE * NA      # 300, e-major (e outer, a inner)
CELLS = CELLS_EE + CELLS_EN  # 1200
RBF_CUT = 8.0
NLAYERS = 2
DT = mybir.dt.float32

_CACHE = {}


def _ap(base, dims):
    """Replace the free dims of a 2D AP with explicit [step, count] dims."""
    return bass.AP(
        tensor=base.tensor,
        offset=base.offset,
        ap=[base.ap[0]] + [[int(s), int(c)] for s, c in dims],
    )


def _build_module():
    nc = bass.Bass()
    AF = mybir.ActivationFunctionType
    MUL = mybir.AluOpType.mult
    ADD = mybir.AluOpType.add

    inp = {}
    def din(name, shape):
        inp[name] = nc.declare_dram_parameter(name, list(shape), DT, isOutput=False)

    din("R", [128, CELLS])
    din("C4", [128, 128])
    din("CNEG2", [128, 1])
    din("WF2_ee", [128, 128])
    din("WF2_en", [128, 128])
    din("BF2_ee", [128, 1])
    din("BF2_en", [128, 1])
    for l in range(NLAYERS):
        din(f"WL2_ee_{l}", [128, 128])
        din(f"WL2_en_{l}", [128, 128])
        din(f"BL2_ee_{l}", [128, 1])
        din(f"BL2_en_{l}", [128, 1])
    din("WR2_ee", [128, 2])
    din("WR2_en", [128, 2])
    din("H0", [128, NSETS * 70])
    din("BRS", [128, 1])
    y = nc.declare_dram_parameter("y", [128, NSETS], DT, isOutput=True)

    groups = [list(range(0, 14)), list(range(14, 28)), list(range(28, 32))]
    NG = len(groups)
    # matmul output chunks must not cross 512-element psum bank boundaries
    ARG_CHUNKS = ((0, 512), (512, 900), (900, 1024), (1024, 1200))

    with contextlib.ExitStack() as st:
        ent = st.enter_context
        block = ent(nc.Block())
        s_dma0 = ent(nc.semaphore("s_dma0"))
        s_rdma = ent(nc.semaphore("s_rdma"))
        s_argmm = ent(nc.semaphore("s_argmm"))
        s_exp = ent(nc.semaphore("s_exp"))
        s_filtmm = ent(nc.semaphore("s_filtmm"))
        s_tanh = ent(nc.semaphore("s_tanh"))
        s_mask = ent(nc.semaphore("s_mask"))
        s_dvemul = ent(nc.semaphore("s_dvemul"))
        s_zmm = ent(nc.semaphore("s_zmm"))
        s_tanh2 = ent(nc.semaphore("s_tanh2"))
        s_hadd = ent(nc.semaphore("s_hadd"))
        s_rs = ent(nc.semaphore("s_rs"))
        s_outmm = ent(nc.semaphore("s_outmm"))
        s_actout = ent(nc.semaphore("s_actout"))
        s_outdma = ent(nc.semaphore("s_outdma"))

        sb = lambda n, sh: ent(nc.sbuf_tensor(n, sh, DT))
        R_t = [sb("R_t0", [4, CELLS]), sb("R_t1", [4, CELLS])]
        C4_t = sb("C4_t", [4, 128])
        CN_t = sb("CN_t", [128, 1])
        WFe_t = sb("WFe_t", [128, 128])
        WFn_t = sb("WFn_t", [128, 128])
        BFe_t = sb("BFe_t", [128, 1])
        BFn_t = sb("BFn_t", [128, 1])
        WL_t = [[sb(f"WLe{l}_t", [128, 128]), sb(f"WLn{l}_t", [128, 128])]
                for l in range(NLAYERS)]
        BL_t = [[sb(f"BLe{l}_t", [128, 1]), sb(f"BLn{l}_t", [128, 1])]
                for l in range(NLAYERS)]
        WRe_t = sb("WRe_t", [128, 2])
        WRn_t = sb("WRn_t", [128, 2])
        BRS_t = sb("BRS_t", [128, 1])
        H_t = sb("H_t", [128, NSETS * 70])
        F_t = sb("F_t", [128, NSETS * CELLS])
        RBF_t = sb("RBF_t", [128, CELLS])
        P_t = sb("P_t", [128, CELLS_EE])
        P2_t = sb("P2_t", [128, CELLS_EN])
        P3_t = sb("P3_t", [128, CELLS_EN])
        AGG_t = sb("AGG_t", [128, 14 * 70])
        T_t = sb("T_t", [128, 14 * 70])
        RS_e = sb("RS_e", [128, NSETS])
        RS_n = sb("RS_n", [128, NSETS])
        O_t = sb("O_t", [2, NSETS])

        ps_arg = ent(nc.psum_tensor("ps_arg", [128, CELLS], DT))
        ps_filt = ent(nc.psum_tensor("ps_filt", [128, CELLS], DT))
        ps_z = ent(nc.psum_tensor("ps_z", [128, 1024], DT))

        n_dma0 = 0

        @block.sync
        def _(sync):
            nonlocal n_dma0
            loads = [
                 (CN_t, inp["CNEG2"]),
                (WFe_t, inp["WF2_ee"]), (WFn_t, inp["WF2_en"]),
                (BFe_t, inp["BF2_ee"]), (BFn_t, inp["BF2_en"]),
                (WRe_t, inp["WR2_ee"]), (WRn_t, inp["WR2_en"]),
                (BRS_t, inp["BRS"]), (H_t, inp["H0"]),
            ]
            for l in range(NLAYERS):
                loads += [(WL_t[l][0], inp[f"WL2_ee_{l}"]),
                          (WL_t[l][1], inp[f"WL2_en_{l}"]),
                          (BL_t[l][0], inp[f"BL2_ee_{l}"]),
                          (BL_t[l][1], inp[f"BL2_en_{l}"])]
            for dst, src in loads:
                sync.dma_start(out=dst[:], in_=src[:, :]).then_inc(s_dma0, 16)
                n_dma0 += 1
            sync.dma_start(out=C4_t[:], in_=inp["C4"][0:4, :]).then_inc(s_dma0, 16)
            n_dma0 += 1
            for s in range(NSETS):
                if s >= 1:
                    sync.wait_ge(s_rdma, 16 * s)
                if s >= 2:
                    sync.wait_ge(s_argmm, s - 1)
                src = bass.AP(tensor=inp["R"], offset=s * CELLS,
                              ap=[[NSETS * CELLS, 4], [1, CELLS]])
                sync.dma_start(out=R_t[s % 2][:], in_=src).then_inc(s_rdma, 16)

        @block.tensor
        def _(tensor):
            tensor.wait_ge(s_dma0, 16 * n_dma0)
            for s in range(NSETS):
                tensor.wait_ge(s_rdma, 16 * (s + 1))
                if s >= 1:
                    tensor.wait_ge(s_exp, s)       # ps_arg consumed by exp
                rt = R_t[s % 2]
                for ci, (c0, c1) in enumerate(ARG_CHUNKS):
                    mm = tensor.matmul(ps_arg[:, c0:c1], C4_t[:], rt[:, c0:c1],
                                       start=True, stop=True)
                    if ci == len(ARG_CHUNKS) - 1:
                        mm.then_inc(s_argmm, 1)
                tensor.wait_ge(s_exp, s + 1)       # rbf ready in SBUF
                if s >= 1:
                    tensor.wait_ge(s_tanh, s)      # ps_filt consumed by tanh
                for ci, (c0, c1) in enumerate(ARG_CHUNKS):
                    w = WFe_t if c1 <= 900 else WFn_t
                    mm = tensor.matmul(ps_filt[:, c0:c1], w[:], RBF_t[:, c0:c1],
                                       start=True, stop=True)
                    if ci == len(ARG_CHUNKS) - 1:
                        mm.then_inc(s_filtmm, 1)
            for l in range(NLAYERS):
                for g, sets in enumerate(groups):
                    gidx = l * NG + g
                    if gidx >= 1:
                        tensor.wait_ge(s_tanh2, 2 * gidx)  # ps_z consumed
                    for q, s in enumerate(sets):
                        tensor.wait_ge(s_dvemul, l * NSETS + s + 1)
                        zoff = 512 * (q // 7) + 70 * (q % 7)
                        tensor.matmul(ps_z[:, zoff:zoff + 30], WL_t[l][0][:],
                                      AGG_t[:, q * 70:q * 70 + 30],
                                      start=True, stop=True)
                        mm = tensor.matmul(ps_z[:, zoff + 30:zoff + 70],
                                           WL_t[l][1][:],
                                           AGG_t[:, q * 70 + 30:q * 70 + 70],
                                           start=True, stop=True)
                        if q == len(sets) - 1:
                            mm.then_inc(s_zmm, 1)
            tensor.wait_ge(s_rs, 2)
            tensor.wait_ge(s_tanh2, 2 * NLAYERS * NG)
            tensor.matmul(ps_z[0:2, 0:NSETS], WRe_t[:], RS_e[:],
                          start=True, stop=False)
            tensor.matmul(ps_z[0:2, 0:NSETS], WRn_t[:], RS_n[:],
                          start=False, stop=True).then_inc(s_outmm, 1)

        @block.scalar
        def _(scalar):
            for s in range(NSETS):
                scalar.wait_ge(s_argmm, s + 1)
                if s >= 1:
                    scalar.wait_ge(s_filtmm, s)    # RBF_t consumed
                scalar.activation(RBF_t[:], ps_arg[:], AF.Exp,
                                  bias=CN_t[:, 0:1],
                                  scale=1.0).then_inc(s_exp, 1)
                scalar.wait_ge(s_filtmm, s + 1)
                fb = s * CELLS
                scalar.activation(F_t[:, fb:fb + 900], ps_filt[:, 0:900],
                                  AF.Tanh, bias=BFe_t[:, 0:1], scale=1.0)
                scalar.activation(F_t[:, fb + 900:fb + 1200],
                                  ps_filt[:, 900:1200],
                                  AF.Tanh, bias=BFn_t[:, 0:1],
                                  scale=1.0).then_inc(s_tanh, 1)
            for l in range(NLAYERS):
                for g, sets in enumerate(groups):
                    gidx = l * NG + g
                    scalar.wait_ge(s_zmm, gidx + 1)
                    if gidx >= 1:
                        scalar.wait_ge(s_hadd, gidx)   # T_t consumed
                    nsg = len(sets)
                    nb = min(nsg, 7)
                    nbank = (nsg + 6) // 7
                    scalar.activation(
                        _ap(T_t[:, 0:1], [[490, nbank], [70, nb], [1, 30]]),
                        _ap(ps_z[:, 0:1], [[512, nbank], [70, nb], [1, 30]]),
                        AF.Tanh, bias=BL_t[l][0][:, 0:1],
                        scale=1.0).then_inc(s_tanh2, 1)
                    scalar.activation(
                        _ap(T_t[:, 30:31], [[490, nbank], [70, nb], [1, 40]]),
                        _ap(ps_z[:, 30:31], [[512, nbank], [70, nb], [1, 40]]),
                        AF.Tanh, bias=BL_t[l][1][:, 0:1],
                        scale=1.0).then_inc(s_tanh2, 1)
            scalar.wait_ge(s_outmm, 1)
            scalar.activation(O_t[:], ps_z[0:2, 0:NSETS], AF.Exp,
                              bias=BRS_t[0:2, 0:1],
                              scale=1.0).then_inc(s_actout, 1)

        @block.vector
        def _(vector):
            for s in range(NSETS):
                vector.wait_ge(s_tanh, s + 1)
                vector.memset(_ap(F_t[:, s * CELLS:s * CELLS + 1], [[31, 30]]),
                              0.0).then_inc(s_mask, 1)
            for l in range(NLAYERS):
                for g, sets in enumerate(groups):
                    gidx = l * NG + g
                    for q, s in enumerate(sets):
                        vector.wait_ge(s_mask, s + 1)
                        if l >= 1:
                            vector.wait_ge(s_hadd, (l - 1) * NG + g + 1)
                        if gidx >= 1:
                            vector.wait_ge(s_zmm, gidx)  # AGG consumed
                        hb = s * 70
                        fb = s * CELLS
                        vector.tensor_mul(
                            _ap(P_t[:, 0:1], [[30, 30], [1, 30]]),
                            _ap(H_t[:, hb:hb + 1], [[0, 30], [1, 30]]),
                            _ap(F_t[:, fb:fb + 1], [[30, 30], [1, 30]]))
                        vector.tensor_reduce(
                            AGG_t[:, q * 70:q * 70 + 30],
                            _ap(P_t[:, 0:1], [[30, 30], [1, 30]]),
                            mybir.AxisListType.X, ADD)
                        vector.tensor_mul(
                            _ap(P2_t[:, 0:1], [[10, 30], [1, 10]]),
                            _ap(H_t[:, hb + 60:hb + 61], [[0, 30], [1, 10]]),
                            _ap(F_t[:, fb + 900:fb + 901], [[10, 30], [1, 10]]))
                        vector.tensor_reduce(
                            AGG_t[:, q * 70 + 30:q * 70 + 60],
                            _ap(P2_t[:, 0:1], [[10, 30], [1, 10]]),
                            mybir.AxisListType.X, ADD)
                        vector.tensor_mul(
                            _ap(P3_t[:, 0:1], [[30, 10], [1, 30]]),
                            _ap(H_t[:, hb + 30:hb + 31], [[0, 10], [1, 30]]),
                            _ap(F_t[:, fb + 900:fb + 901], [[1, 10], [10, 30]]))
                        vector.tensor_reduce(
                            AGG_t[:, q * 70 + 60:q * 70 + 70],
                            _ap(P3_t[:, 0:1], [[30, 10], [1, 30]]),
                            mybir.AxisListType.X, ADD).then_inc(s_dvemul, 1)
                    vector.wait_ge(s_tanh2, 2 * (gidx + 1))
                    nsg = len(sets)
                    hslice = H_t[:, sets[0] * 70:(sets[-1] + 1) * 70]
                    vector.tensor_add(hslice, hslice,
                                      T_t[:, 0:nsg * 70]).then_inc(s_hadd, 1)
            vector.wait_ge(s_hadd, NLAYERS * NG)
            vector.tensor_reduce(RS_e[:],
                                 _ap(H_t[:, 0:1], [[70, NSETS], [1, 30]]),
                                 mybir.AxisListType.X, ADD).then_inc(s_rs, 1)
            vector.tensor_reduce(RS_n[:],
                                 _ap(H_t[:, 30:31], [[70, NSETS], [1, 40]]),
                                 mybir.AxisListType.X, ADD).then_inc(s_rs, 1)

        @block.gpsimd
        def _(gpsimd):
            gpsimd.wait_ge(s_actout, 1)
            gpsimd.dma_start(out=y[0:2, :], in_=O_t[:]).then_inc(s_outdma, 16)
            gpsimd.wait_ge(s_outdma, 16)

    return nc


def _host_prep(pos, atoms, emb_ee, wf_ee, bf_ee, wl_ee, bl_ee, wr_ee, br_ee,
               emb_en, wf_en, bf_en, wl_en, bl_en, wr_en, br_en,
               ee_types, en_types):
    f32 = np.float32
    centers = np.linspace(0.0, RBF_CUT, K).astype(f32)

    xyz = pos.reshape(NB, NE, 3).astype(f32)
    diff = xyz[:, None, :, :] - xyz[:, :, None, :]        # [nb, j, i, 3]
    d_ee = np.sqrt((diff ** 2).sum(-1)).reshape(NB, CELLS_EE)
    dn = xyz[:, :, None, :] - atoms.astype(f32)[None, None, :, :]
    d_en = np.sqrt((dn ** 2).sum(-1)).reshape(NB, CELLS_EN)
    d = np.concatenate([d_ee, d_en], axis=1).astype(f32)
    dsq = d * d

    def blockdiag(w):
        o = np.zeros((128, 128), f32)
        o[:64, :64] = w
        o[64:, 64:] = w
        return o

    C4 = np.zeros((128, 128), f32)
    C4[0, :64] = -1.0
    C4[1, :64] = 2.0 * centers
    C4[2, 64:] = -1.0
    C4[3, 64:] = 2.0 * centers
    CNEG2 = np.tile(-(centers ** 2), 2).reshape(128, 1).astype(f32)

    def rep2(v):
        return np.tile(np.asarray(v, f32).reshape(-1), 2).reshape(128, 1)

    h0_ee = emb_ee[ee_types].T.astype(f32)
    h0_en = emb_en[en_types].T.astype(f32)
    H0_half = np.concatenate([h0_ee, h0_en], axis=1)
    H0_one = np.concatenate([H0_half, H0_half], axis=0)
    H0 = np.tile(H0_one[:, None, :], (1, NSETS, 1)).reshape(128, NSETS * 70)

    WR2_ee = np.zeros((128, 2), f32)
    WR2_ee[:64, 0] = wr_ee[:, 0]
    WR2_ee[64:, 1] = wr_ee[:, 0]
    WR2_en = np.zeros((128, 2), f32)
    WR2_en[:64, 0] = wr_en[:, 0]
    WR2_en[64:, 1] = wr_en[:, 0]

    const = {
        "C4": C4, "CNEG2": CNEG2,
        "WF2_ee": blockdiag(wf_ee), "WF2_en": blockdiag(wf_en),
        "BF2_ee": rep2(bf_ee), "BF2_en": rep2(bf_en),
        "WR2_ee": WR2_ee, "WR2_en": WR2_en, "H0": np.ascontiguousarray(H0),
        "BRS": np.full((128, 1), float(br_ee[0]) + float(br_en[0]), f32),
    }
    for l in range(NLAYERS):
        const[f"WL2_ee_{l}"] = blockdiag(wl_ee[l])
        const[f"WL2_en_{l}"] = blockdiag(wl_en[l])
        const[f"BL2_ee_{l}"] = rep2(bl_ee[l])
        const[f"BL2_en_{l}"] = rep2(bl_en[l])

    in_maps = []
    for c in range(N_CORES):
        dloc = d[c * NW:(c + 1) * NW]
        dsloc = dsq[c * NW:(c + 1) * NW]
        R = np.empty((4, NSETS, CELLS), f32)
        R[0] = dsloc[0::2]
        R[1] = dloc[0::2]
        R[2] = dsloc[1::2]
        R[3] = dloc[1::2]
        m = dict(const)
        m["R"] = np.ascontiguousarray(R.reshape(128, CELLS))
        in_maps.append(m)
    return in_maps


def kernel(pos, atoms, emb_ee, wf_ee, bf_ee, wl_ee, bl_ee, wr_ee, br_ee,
           emb_en, wf_en, bf_en, wl_en, bl_en, wr_en, br_en,
           ee_src, ee_dst, ee_types, en_src, en_dst, en_types):
    in_maps = _host_prep(
        np.asarray(pos), np.asarray(atoms), np.asarray(emb_ee),
        np.asarray(wf_ee), np.asarray(bf_ee), np.asarray(wl_ee),
        np.asarray(bl_ee), np.asarray(wr_ee), np.asarray(br_ee),
        np.asarray(emb_en), np.asarray(wf_en), np.asarray(bf_en),
        np.asarray(wl_en), np.asarray(bl_en), np.asarray(wr_en),
        np.asarray(br_en), np.asarray(ee_types), np.asarray(en_types))
    if "nc" not in _CACHE:
        _CACHE["nc"] = _build_module()
    res = run_bass_kernel_spmd(_CACHE["nc"], in_maps, list(range(N_CORES)))
    out = np.concatenate(
        [res.results[c]["y"][0:2, :].T.reshape(NW, 1) for c in range(N_CORES)],
        axis=0)
    return out.astype(np.float32)



# revision 3
# speedup vs baseline: 1.3763x; 1.3763x over previous
"""Trainium2 Bass kernel for nn_JastrowFactorGraph (MGCN-style Jastrow factor).

Strategy (8 NeuronCores, pure data parallel over the 512-walker batch):
  - 64 walkers per core; SBUF layout uses 128 partitions = 64 features x 2
    walkers ("walker-set"); 32 sets per core processed as batched phases.
  - Dense symmetric filter grids: e-e is the complete graph on 30 electrons
    (30x30 grid, diag zeroed), e-n is bipartite (30x10 grid used in both
    directions). Gather/scatter become broadcast-AP multiplies + free-dim
    reductions on the vector engine.
  - RBF expansion exp(-(d-c_k)^2) is computed as exp(PE_matmul + bias):
    a K=4 matmul with rows {-dsq, 2c*d} per walker slot and the -c_k^2 term
    folded into the activation bias (per-partition vector).
  - filt = tanh(rbf @ wf + bf) and layer updates h += tanh(agg @ wl + bl)
    are block-diagonal PE matmuls contracting the feature dim on partitions.
  - Readout sum_n h[n] @ wr is a free-dim reduce + one K=128 matmul; the two
    graph scalars accumulate in PSUM and exp() is applied on-chip.
"""

import contextlib

import numpy as np

import concourse.bass as bass
import concourse.mybir as mybir
from concourse.bass_utils import run_bass_kernel_spmd

N_CORES = 8
NB = 512
NW = NB // N_CORES      # walkers per core = 64
NSETS = NW // 2         # 32 walker-sets (2 walkers per set)
NE = 30                 # electrons (ee nodes)
NA = 10                 # atoms
F = 64                  # features
K = 64                  # RBF size
CELLS_EE = NE * NE      # 900, j-major (j outer, i inner)
CELLS_EN = NE * NA      # 300, e-major (e outer, a inner)
CELLS = CELLS_EE + CELLS_EN  # 1200
RBF_CUT = 8.0
NLAYERS = 2
DT = mybir.dt.float32
DTR = mybir.dt.float32r

_CACHE = {}


def _ap(base, dims):
    """Replace the free dims of a 2D AP with explicit [step, count] dims."""
    return bass.AP(
        tensor=base.tensor,
        offset=base.offset,
        ap=[base.ap[0]] + [[int(s), int(c)] for s, c in dims],
    )


def _build_module():
    nc = bass.Bass()
    AF = mybir.ActivationFunctionType
    MUL = mybir.AluOpType.mult
    ADD = mybir.AluOpType.add

    inp = {}
    def din(name, shape):
        inp[name] = nc.declare_dram_parameter(name, list(shape), DT, isOutput=False)

    inp["R"] = nc.declare_dram_parameter("R", [10, NSETS * CELLS], DTR, isOutput=False)
    inp["C4"] = nc.declare_dram_parameter("C4", [10, 128], DTR, isOutput=False)
    din("CNEG2", [128, 1])
    inp["WF2_ee"] = nc.declare_dram_parameter("WF2_ee", [128, 128], DTR, isOutput=False)
    inp["WF2_en"] = nc.declare_dram_parameter("WF2_en", [128, 128], DTR, isOutput=False)
    din("BF2_ee", [128, 1])
    din("BF2_en", [128, 1])
    for l in range(NLAYERS):
        din(f"WL2_ee_{l}", [128, 128])
        din(f"WL2_en_{l}", [128, 128])
        din(f"BL2_ee_{l}", [128, 1])
        din(f"BL2_en_{l}", [128, 1])
    din("WR2_ee", [128, 2])
    din("WR2_en", [128, 2])
    din("H0", [128, NSETS * 70])
    din("BRS", [128, 1])
    y = nc.declare_dram_parameter("y", [128, NSETS], DT, isOutput=True)

    groups = [list(range(0, 14)), list(range(14, 28)), list(range(28, 32))]
    NG = len(groups)
    # matmul output chunks must not cross 512-element psum bank boundaries
    ARG_CHUNKS = ((0, 512), (512, 900), (900, 1024), (1024, 1200))

    with contextlib.ExitStack() as st:
        ent = st.enter_context
        block = ent(nc.Block())
        s_dma0 = ent(nc.semaphore("s_dma0"))
        s_rdma = ent(nc.semaphore("s_rdma"))
        s_argmm = ent(nc.semaphore("s_argmm"))
        s_exp = ent(nc.semaphore("s_exp"))
        s_filtmm = ent(nc.semaphore("s_filtmm"))
        s_tanh = ent(nc.semaphore("s_tanh"))
        s_mask = ent(nc.semaphore("s_mask"))
        s_dvemul = ent(nc.semaphore("s_dvemul"))
        s_zmm = ent(nc.semaphore("s_zmm"))
        s_tanh2 = ent(nc.semaphore("s_tanh2"))
        s_hadd = ent(nc.semaphore("s_hadd"))
        s_rs = ent(nc.semaphore("s_rs"))
        s_outmm = ent(nc.semaphore("s_outmm"))
        s_actout = ent(nc.semaphore("s_actout"))
        s_outdma = ent(nc.semaphore("s_outdma"))

        sb = lambda n, sh: ent(nc.sbuf_tensor(n, sh, DT))
        R_t = [ent(nc.sbuf_tensor("R_t0", [10, CELLS], DTR)), ent(nc.sbuf_tensor("R_t1", [10, CELLS], DTR))]
        C4_t = ent(nc.sbuf_tensor("C4_t", [10, 128], DTR))
        CN_t = sb("CN_t", [128, 1])
        WFe_t = ent(nc.sbuf_tensor("WFe_t", [128, 128], DTR))
        WFn_t = ent(nc.sbuf_tensor("WFn_t", [128, 128], DTR))
        BFe_t = sb("BFe_t", [128, 1])
        BFn_t = sb("BFn_t", [128, 1])
        WL_t = [[sb(f"WLe{l}_t", [128, 128]), sb(f"WLn{l}_t", [128, 128])]
                for l in range(NLAYERS)]
        BL_t = [[sb(f"BLe{l}_t", [128, 1]), sb(f"BLn{l}_t", [128, 1])]
                for l in range(NLAYERS)]
        WRe_t = sb("WRe_t", [128, 2])
        WRn_t = sb("WRn_t", [128, 2])
        BRS_t = sb("BRS_t", [128, 1])
        H_t = sb("H_t", [128, NSETS * 70])
        F_t = sb("F_t", [128, NSETS * CELLS])
        RBF_t = ent(nc.sbuf_tensor("RBF_t", [128, CELLS], DTR))
        P_t = sb("P_t", [128, CELLS_EE])
        P2_t = sb("P2_t", [128, CELLS_EN])
        P3_t = sb("P3_t", [128, CELLS_EN])
        AGG_t = sb("AGG_t", [128, 14 * 70])
        T_t = sb("T_t", [128, 14 * 70])
        RS_e = sb("RS_e", [128, NSETS])
        RS_n = sb("RS_n", [128, NSETS])
        O_t = sb("O_t", [2, NSETS])

        ps_arg = ent(nc.psum_tensor("ps_arg", [128, CELLS], DT))
        ps_filt = ent(nc.psum_tensor("ps_filt", [128, CELLS], DT))
        ps_z = ent(nc.psum_tensor("ps_z", [128, 1024], DT))

        n_dma0 = 0

        @block.sync
        def _(sync):
            nonlocal n_dma0
            loads = [
                 (CN_t, inp["CNEG2"]),
                (WFe_t, inp["WF2_ee"]), (WFn_t, inp["WF2_en"]),
                (BFe_t, inp["BF2_ee"]), (BFn_t, inp["BF2_en"]),
                (WRe_t, inp["WR2_ee"]), (WRn_t, inp["WR2_en"]),
                (BRS_t, inp["BRS"]), (H_t, inp["H0"]),
            ]
            for l in range(NLAYERS):
                loads += [(WL_t[l][0], inp[f"WL2_ee_{l}"]),
                          (WL_t[l][1], inp[f"WL2_en_{l}"]),
                          (BL_t[l][0], inp[f"BL2_ee_{l}"]),
                          (BL_t[l][1], inp[f"BL2_en_{l}"])]
            for dst, src in loads:
                sync.dma_start(out=dst[:], in_=src[:, :]).then_inc(s_dma0, 16)
                n_dma0 += 1
            sync.dma_start(out=C4_t[:], in_=inp["C4"][0:10, :]).then_inc(s_dma0, 16)
            n_dma0 += 1
            for s in range(NSETS):
                if s >= 1:
                    sync.wait_ge(s_rdma, 16 * s)
                if s >= 2:
                    sync.wait_ge(s_argmm, s - 1)
                src = bass.AP(tensor=inp["R"], offset=s * CELLS,
                              ap=[[NSETS * CELLS, 10], [1, CELLS]])
                sync.dma_start(out=R_t[s % 2][:], in_=src).then_inc(s_rdma, 16)

        @block.tensor
        def _(tensor):
            tensor.wait_ge(s_dma0, 16 * n_dma0)
            for s in range(NSETS):
                tensor.wait_ge(s_rdma, 16 * (s + 1))
                if s >= 1:
                    tensor.wait_ge(s_exp, s)       # ps_arg consumed by exp
                rt = R_t[s % 2]
                for ci, (c0, c1) in enumerate(ARG_CHUNKS):
                    mm = tensor.matmul(ps_arg[:, c0:c1], C4_t[:], rt[:, c0:c1],
                                       start=True, stop=True)
                    if ci == len(ARG_CHUNKS) - 1:
                        mm.then_inc(s_argmm, 1)
                tensor.wait_ge(s_exp, s + 1)       # rbf ready in SBUF
                if s >= 1:
                    tensor.wait_ge(s_tanh, s)      # ps_filt consumed by tanh
                for ci, (c0, c1) in enumerate(ARG_CHUNKS):
                    w = WFe_t if c1 <= 900 else WFn_t
                    mm = tensor.matmul(ps_filt[:, c0:c1], w[:], RBF_t[:, c0:c1],
                                       start=True, stop=True)
                    if ci == len(ARG_CHUNKS) - 1:
                        mm.then_inc(s_filtmm, 1)
            for l in range(NLAYERS):
                for g, sets in enumerate(groups):
                    gidx = l * NG + g
                    if gidx >= 1:
                        tensor.wait_ge(s_tanh2, 2 * gidx)  # ps_z consumed
                    for q, s in enumerate(sets):
                        tensor.wait_ge(s_dvemul, l * NSETS + s + 1)
                        zoff = 512 * (q // 7) + 70 * (q % 7)
                        tensor.matmul(ps_z[:, zoff:zoff + 30], WL_t[l][0][:],
                                      AGG_t[:, q * 70:q * 70 + 30],
                                      start=True, stop=True)
                        mm = tensor.matmul(ps_z[:, zoff + 30:zoff + 70],
                                           WL_t[l][1][:],
                                           AGG_t[:, q * 70 + 30:q * 70 + 70],
                                           start=True, stop=True)
                        if q == len(sets) - 1:
                            mm.then_inc(s_zmm, 1)
            tensor.wait_ge(s_rs, 2)
            tensor.wait_ge(s_tanh2, 2 * NLAYERS * NG)
            tensor.matmul(ps_z[0:2, 0:NSETS], WRe_t[:], RS_e[:],
                          start=True, stop=False)
            tensor.matmul(ps_z[0:2, 0:NSETS], WRn_t[:], RS_n[:],
                          start=False, stop=True).then_inc(s_outmm, 1)

        @block.scalar
        def _(scalar):
            for s in range(NSETS):
                scalar.wait_ge(s_argmm, s + 1)
                if s >= 1:
                    scalar.wait_ge(s_filtmm, s)    # RBF_t consumed
                scalar.activation(RBF_t[:], ps_arg[:], AF.Exp,
                                  bias=CN_t[:, 0:1],
                                  scale=1.0).then_inc(s_exp, 1)
                scalar.wait_ge(s_filtmm, s + 1)
                fb = s * CELLS
                scalar.activation(F_t[:, fb:fb + 900], ps_filt[:, 0:900],
                                  AF.Tanh, bias=BFe_t[:, 0:1], scale=1.0)
                scalar.activation(F_t[:, fb + 900:fb + 1200],
                                  ps_filt[:, 900:1200],
                                  AF.Tanh, bias=BFn_t[:, 0:1],
                                  scale=1.0).then_inc(s_tanh, 1)
            for l in range(NLAYERS):
                for g, sets in enumerate(groups):
                    gidx = l * NG + g
                    scalar.wait_ge(s_zmm, gidx + 1)
                    if gidx >= 1:
                        scalar.wait_ge(s_hadd, gidx)   # T_t consumed
                    nsg = len(sets)
                    nb = min(nsg, 7)
                    nbank = (nsg + 6) // 7
                    scalar.activation(
                        _ap(T_t[:, 0:1], [[490, nbank], [70, nb], [1, 30]]),
                        _ap(ps_z[:, 0:1], [[512, nbank], [70, nb], [1, 30]]),
                        AF.Tanh, bias=BL_t[l][0][:, 0:1],
                        scale=1.0).then_inc(s_tanh2, 1)
                    scalar.activation(
                        _ap(T_t[:, 30:31], [[490, nbank], [70, nb], [1, 40]]),
                        _ap(ps_z[:, 30:31], [[512, nbank], [70, nb], [1, 40]]),
                        AF.Tanh, bias=BL_t[l][1][:, 0:1],
                        scale=1.0).then_inc(s_tanh2, 1)
            scalar.wait_ge(s_outmm, 1)
            scalar.activation(O_t[:], ps_z[0:2, 0:NSETS], AF.Exp,
                              bias=BRS_t[0:2, 0:1],
                              scale=1.0).then_inc(s_actout, 1)

        @block.vector
        def _(vector):
            for s in range(NSETS):
                vector.wait_ge(s_tanh, s + 1)
                vector.memset(_ap(F_t[:, s * CELLS:s * CELLS + 1], [[31, 30]]),
                              0.0).then_inc(s_mask, 1)
            for l in range(NLAYERS):
                for g, sets in enumerate(groups):
                    gidx = l * NG + g
                    for q, s in enumerate(sets):
                        vector.wait_ge(s_mask, s + 1)
                        if l >= 1:
                            vector.wait_ge(s_hadd, (l - 1) * NG + g + 1)
                        if gidx >= 1:
                            vector.wait_ge(s_zmm, gidx)  # AGG consumed
                        hb = s * 70
                        fb = s * CELLS
                        vector.tensor_mul(
                            _ap(P_t[:, 0:1], [[30, 30], [1, 30]]),
                            _ap(H_t[:, hb:hb + 1], [[0, 30], [1, 30]]),
                            _ap(F_t[:, fb:fb + 1], [[30, 30], [1, 30]]))
                        vector.tensor_reduce(
                            AGG_t[:, q * 70:q * 70 + 30],
                            _ap(P_t[:, 0:1], [[30, 30], [1, 30]]),
                            mybir.AxisListType.X, ADD)
                        vector.tensor_mul(
                            _ap(P2_t[:, 0:1], [[10, 30], [1, 10]]),
                            _ap(H_t[:, hb + 60:hb + 61], [[0, 30], [1, 10]]),
                            _ap(F_t[:, fb + 900:fb + 901], [[10, 30], [1, 10]]))
                        vector.tensor_reduce(
                            AGG_t[:, q * 70 + 30:q * 70 + 60],
                            _ap(P2_t[:, 0:1], [[10, 30], [1, 10]]),
                            mybir.AxisListType.X, ADD)
                        vector.tensor_mul(
                            _ap(P3_t[:, 0:1], [[30, 10], [1, 30]]),
                            _ap(H_t[:, hb + 30:hb + 31], [[0, 10], [1, 30]]),
                            _ap(F_t[:, fb + 900:fb + 901], [[1, 10], [10, 30]]))
                        vector.tensor_reduce(
                            AGG_t[:, q * 70 + 60:q * 70 + 70],
                            _ap(P3_t[:, 0:1], [[30, 10], [1, 30]]),
                            mybir.AxisListType.X, ADD).then_inc(s_dvemul, 1)
                    vector.wait_ge(s_tanh2, 2 * (gidx + 1))
                    nsg = len(sets)
                    hslice = H_t[:, sets[0] * 70:(sets[-1] + 1) * 70]
                    vector.tensor_add(hslice, hslice,
                                      T_t[:, 0:nsg * 70]).then_inc(s_hadd, 1)
            vector.wait_ge(s_hadd, NLAYERS * NG)
            vector.tensor_reduce(RS_e[:],
                                 _ap(H_t[:, 0:1], [[70, NSETS], [1, 30]]),
                                 mybir.AxisListType.X, ADD).then_inc(s_rs, 1)
            vector.tensor_reduce(RS_n[:],
                                 _ap(H_t[:, 30:31], [[70, NSETS], [1, 40]]),
                                 mybir.AxisListType.X, ADD).then_inc(s_rs, 1)

        @block.gpsimd
        def _(gpsimd):
            gpsimd.wait_ge(s_actout, 1)
            gpsimd.dma_start(out=y[0:2, :], in_=O_t[:]).then_inc(s_outdma, 16)
            gpsimd.wait_ge(s_outdma, 16)

    return nc


def _host_prep(pos, atoms, emb_ee, wf_ee, bf_ee, wl_ee, bl_ee, wr_ee, br_ee,
               emb_en, wf_en, bf_en, wl_en, bl_en, wr_en, br_en,
               ee_types, en_types):
    f32 = np.float32
    centers = np.linspace(0.0, RBF_CUT, K).astype(f32)

    xyz = pos.astype(np.float64).reshape(NB, NE, 3)
    diff = xyz[:, None, :, :] - xyz[:, :, None, :]        # [nb, j, i, 3]
    d_ee = np.sqrt((diff ** 2).sum(-1)).reshape(NB, CELLS_EE)
    dn = xyz[:, :, None, :] - atoms.astype(np.float64)[None, None, :, :]
    d_en = np.sqrt((dn ** 2).sum(-1)).reshape(NB, CELLS_EN)
    d = np.concatenate([d_ee, d_en], axis=1)
    dsq = d * d
    dsq_hi = dsq.astype(np.float16).astype(np.float64)
    dsq_lo = (dsq - dsq_hi).astype(f32)
    d_hi = d.astype(np.float16).astype(np.float64)
    d_lo = (d - d_hi).astype(f32)
    dsq_hi = dsq_hi.astype(f32)
    d_hi = d_hi.astype(f32)

    def blockdiag(w):
        o = np.zeros((128, 128), f32)
        o[:64, :64] = w
        o[64:, 64:] = w
        return o

    chi = (2.0 * centers).astype(np.float16).astype(f32)
    clo = (2.0 * centers.astype(np.float64) - chi).astype(f32)
    C4 = np.zeros((10, 128), f32)
    for w in range(2):
        C4[5 * w + 0, 64 * w:64 * w + 64] = -1.0
        C4[5 * w + 1, 64 * w:64 * w + 64] = -1.0
        C4[5 * w + 2, 64 * w:64 * w + 64] = chi
        C4[5 * w + 3, 64 * w:64 * w + 64] = clo
        C4[5 * w + 4, 64 * w:64 * w + 64] = chi
    CNEG2 = np.tile(-(centers ** 2), 2).reshape(128, 1).astype(f32)

    def rep2(v):
        return np.tile(np.asarray(v, f32).reshape(-1), 2).reshape(128, 1)

    h0_ee = emb_ee[ee_types].T.astype(f32)
    h0_en = emb_en[en_types].T.astype(f32)
    H0_half = np.concatenate([h0_ee, h0_en], axis=1)
    H0_one = np.concatenate([H0_half, H0_half], axis=0)
    H0 = np.tile(H0_one[:, None, :], (1, NSETS, 1)).reshape(128, NSETS * 70)

    WR2_ee = np.zeros((128, 2), f32)
    WR2_ee[:64, 0] = wr_ee[:, 0]
    WR2_ee[64:, 1] = wr_ee[:, 0]
    WR2_en = np.zeros((128, 2), f32)
    WR2_en[:64, 0] = wr_en[:, 0]
    WR2_en[64:, 1] = wr_en[:, 0]

    const = {
        "C4": C4, "CNEG2": CNEG2,
        "WF2_ee": blockdiag(wf_ee), "WF2_en": blockdiag(wf_en),
        "BF2_ee": rep2(bf_ee), "BF2_en": rep2(bf_en),
        "WR2_ee": WR2_ee, "WR2_en": WR2_en, "H0": np.ascontiguousarray(H0),
        "BRS": np.full((128, 1), float(br_ee[0]) + float(br_en[0]), f32),
    }
    for l in range(NLAYERS):
        const[f"WL2_ee_{l}"] = blockdiag(wl_ee[l])
        const[f"WL2_en_{l}"] = blockdiag(wl_en[l])
        const[f"BL2_ee_{l}"] = rep2(bl_ee[l])
        const[f"BL2_en_{l}"] = rep2(bl_en[l])

    in_maps = []
    for c in range(N_CORES):
        lo = c * NW
        R = np.empty((10, NSETS, CELLS), f32)
        for w in range(2):
            R[5 * w + 0] = dsq_hi[lo + w::2][:NSETS]
            R[5 * w + 1] = dsq_lo[lo + w::2][:NSETS]
            R[5 * w + 2] = d_hi[lo + w::2][:NSETS]
            R[5 * w + 3] = d_hi[lo + w::2][:NSETS]
            R[5 * w + 4] = d_lo[lo + w::2][:NSETS]
        m = dict(const)
        m["R"] = np.ascontiguousarray(R.reshape(10, NSETS * CELLS))
        in_maps.append(m)
    return in_maps


def kernel(pos, atoms, emb_ee, wf_ee, bf_ee, wl_ee, bl_ee, wr_ee, br_ee,
           emb_en, wf_en, bf_en, wl_en, bl_en, wr_en, br_en,
           ee_src, ee_dst, ee_types, en_src, en_dst, en_types):
    in_maps = _host_prep(
        np.asarray(pos), np.asarray(atoms), np.asarray(emb_ee),
        np.asarray(wf_ee), np.asarray(bf_ee), np.asarray(wl_ee),
        np.asarray(bl_ee), np.asarray(wr_ee), np.asarray(br_ee),
        np.asarray(emb_en), np.asarray(wf_en), np.asarray(bf_en),
        np.asarray(wl_en), np.asarray(bl_en), np.asarray(wr_en),
        np.asarray(br_en), np.asarray(ee_types), np.asarray(en_types))
    if "nc" not in _CACHE:
        _CACHE["nc"] = _build_module()
    res = run_bass_kernel_spmd(_CACHE["nc"], in_maps, list(range(N_CORES)))
    out = np.concatenate(
        [res.results[c]["y"][0:2, :].T.reshape(NW, 1) for c in range(N_CORES)],
        axis=0)
    return out.astype(np.float32)



# revision 4
# speedup vs baseline: 1.5436x; 1.1215x over previous
"""Trainium2 Bass kernel for nn_JastrowFactorGraph (MGCN-style Jastrow factor).

Strategy (8 NeuronCores, pure data parallel over the 512-walker batch):
  - 64 walkers per core; SBUF layout uses 128 partitions = 64 features x 2
    walkers ("walker-set"); 32 sets per core processed as batched phases.
  - Dense symmetric filter grids: e-e is the complete graph on 30 electrons
    (30x30 grid, diag zeroed), e-n is bipartite (30x10 grid used in both
    directions). Gather/scatter become broadcast-AP multiplies + free-dim
    reductions on the vector engine.
  - RBF expansion exp(-(d-c_k)^2) is computed as exp(PE_matmul + bias):
    a K=4 matmul with rows {-dsq, 2c*d} per walker slot and the -c_k^2 term
    folded into the activation bias (per-partition vector).
  - filt = tanh(rbf @ wf + bf) and layer updates h += tanh(agg @ wl + bl)
    are block-diagonal PE matmuls contracting the feature dim on partitions.
  - Readout sum_n h[n] @ wr is a free-dim reduce + one K=128 matmul; the two
    graph scalars accumulate in PSUM and exp() is applied on-chip.
"""

import contextlib

import numpy as np

import concourse.bass as bass
import concourse.mybir as mybir
from concourse.bass_utils import run_bass_kernel_spmd

N_CORES = 8
NB = 512
NW = NB // N_CORES      # walkers per core = 64
NSETS = NW // 2         # 32 walker-sets (2 walkers per set)
NE = 30                 # electrons (ee nodes)
NA = 10                 # atoms
F = 64                  # features
K = 64                  # RBF size
CELLS_EE = NE * NE      # 900, j-major (j outer, i inner)
CELLS_EN = NE * NA      # 300, e-major (e outer, a inner)
CELLS = CELLS_EE + CELLS_EN  # 1200
RBF_CUT = 8.0
NLAYERS = 2
DT = mybir.dt.float32
DTR = mybir.dt.float32r
DT16 = mybir.dt.float16

_CACHE = {}


def _ap(base, dims):
    """Replace the free dims of a 2D AP with explicit [step, count] dims."""
    return bass.AP(
        tensor=base.tensor,
        offset=base.offset,
        ap=[base.ap[0]] + [[int(s), int(c)] for s, c in dims],
    )


def _build_module():
    nc = bass.Bass()
    AF = mybir.ActivationFunctionType
    MUL = mybir.AluOpType.mult
    ADD = mybir.AluOpType.add

    inp = {}
    def din(name, shape):
        inp[name] = nc.declare_dram_parameter(name, list(shape), DT, isOutput=False)

    inp["R"] = nc.declare_dram_parameter("R", [10, NSETS * CELLS], DTR, isOutput=False)
    inp["C4"] = nc.declare_dram_parameter("C4", [10, 128], DTR, isOutput=False)
    din("CNEG2", [128, 1])
    inp["WF2_ee"] = nc.declare_dram_parameter("WF2_ee", [128, 128], DTR, isOutput=False)
    inp["WF2_en"] = nc.declare_dram_parameter("WF2_en", [128, 128], DTR, isOutput=False)
    din("BF2_ee", [128, 1])
    din("BF2_en", [128, 1])
    for l in range(NLAYERS):
        inp[f"WL2_ee_{l}"] = nc.declare_dram_parameter(f"WL2_ee_{l}", [128, 128], DT16, isOutput=False)
        inp[f"WL2_en_{l}"] = nc.declare_dram_parameter(f"WL2_en_{l}", [128, 128], DT16, isOutput=False)
        din(f"BL2_ee_{l}", [128, 1])
        din(f"BL2_en_{l}", [128, 1])
    inp["WR2_ee"] = nc.declare_dram_parameter("WR2_ee", [128, 2], DT16, isOutput=False)
    inp["WR2_en"] = nc.declare_dram_parameter("WR2_en", [128, 2], DT16, isOutput=False)
    inp["H0"] = nc.declare_dram_parameter("H0", [128, NSETS * 70], DT16, isOutput=False)
    din("BRS", [128, 1])
    y = nc.declare_dram_parameter("y", [128, NSETS], DT, isOutput=True)

    groups = [list(range(0, 14)), list(range(14, 28)), list(range(28, 32))]
    NG = len(groups)
    # matmul output chunks must not cross 512-element psum bank boundaries
    ARG_CHUNKS = ((0, 512), (512, 900), (900, 1024), (1024, 1200))

    with contextlib.ExitStack() as st:
        ent = st.enter_context
        block = ent(nc.Block())
        ent(nc.allow_low_precision(reason="fp16 aggregation"))
        s_dma0 = ent(nc.semaphore("s_dma0"))
        s_rdma = ent(nc.semaphore("s_rdma"))
        s_argmm = ent(nc.semaphore("s_argmm"))
        s_exp = ent(nc.semaphore("s_exp"))
        s_filtmm = ent(nc.semaphore("s_filtmm"))
        s_tanh = ent(nc.semaphore("s_tanh"))
        s_mask = ent(nc.semaphore("s_mask"))
        s_dvemul = ent(nc.semaphore("s_dvemul"))
        s_zmm = ent(nc.semaphore("s_zmm"))
        s_tanh2 = ent(nc.semaphore("s_tanh2"))
        s_hadd = ent(nc.semaphore("s_hadd"))
        s_rs = ent(nc.semaphore("s_rs"))
        s_outmm = ent(nc.semaphore("s_outmm"))
        s_actout = ent(nc.semaphore("s_actout"))
        s_outdma = ent(nc.semaphore("s_outdma"))

        sb = lambda n, sh: ent(nc.sbuf_tensor(n, sh, DT))
        R_t = [ent(nc.sbuf_tensor("R_t0", [10, CELLS], DTR)), ent(nc.sbuf_tensor("R_t1", [10, CELLS], DTR))]
        C4_t = ent(nc.sbuf_tensor("C4_t", [10, 128], DTR))
        CN_t = sb("CN_t", [128, 1])
        WFe_t = ent(nc.sbuf_tensor("WFe_t", [128, 128], DTR))
        WFn_t = ent(nc.sbuf_tensor("WFn_t", [128, 128], DTR))
        BFe_t = sb("BFe_t", [128, 1])
        BFn_t = sb("BFn_t", [128, 1])
        WL_t = [[ent(nc.sbuf_tensor(f"WLe{l}_t", [128, 128], DT16)), ent(nc.sbuf_tensor(f"WLn{l}_t", [128, 128], DT16))]
                for l in range(NLAYERS)]
        BL_t = [[sb(f"BLe{l}_t", [128, 1]), sb(f"BLn{l}_t", [128, 1])]
                for l in range(NLAYERS)]
        WRe_t = ent(nc.sbuf_tensor("WRe_t", [128, 2], DT16))
        WRn_t = ent(nc.sbuf_tensor("WRn_t", [128, 2], DT16))
        BRS_t = sb("BRS_t", [128, 1])
        H_t = ent(nc.sbuf_tensor("H_t", [128, NSETS * 70], DT16))
        F_t = ent(nc.sbuf_tensor("F_t", [128, NSETS * CELLS], DT16))
        RBF_t = ent(nc.sbuf_tensor("RBF_t", [128, CELLS], DTR))
        P_t = ent(nc.sbuf_tensor("P_t", [128, CELLS_EE], DT16))
        P2_t = ent(nc.sbuf_tensor("P2_t", [128, CELLS_EN], DT16))
        P3_t = ent(nc.sbuf_tensor("P3_t", [128, CELLS_EN], DT16))
        AGG_t = ent(nc.sbuf_tensor("AGG_t", [128, 14 * 70], DT16))
        T_t = ent(nc.sbuf_tensor("T_t", [128, 14 * 70], DT16))
        RS_e = ent(nc.sbuf_tensor("RS_e", [128, NSETS], DT16))
        RS_n = ent(nc.sbuf_tensor("RS_n", [128, NSETS], DT16))
        O_t = sb("O_t", [2, NSETS])

        ps_arg = ent(nc.psum_tensor("ps_arg", [128, CELLS], DT))
        ps_filt = ent(nc.psum_tensor("ps_filt", [128, CELLS], DT))
        ps_z = ent(nc.psum_tensor("ps_z", [128, 1024], DT))

        n_dma0 = 0

        @block.sync
        def _(sync):
            nonlocal n_dma0
            loads = [
                 (CN_t, inp["CNEG2"]),
                (WFe_t, inp["WF2_ee"]), (WFn_t, inp["WF2_en"]),
                (BFe_t, inp["BF2_ee"]), (BFn_t, inp["BF2_en"]),
                (WRe_t, inp["WR2_ee"]), (WRn_t, inp["WR2_en"]),
                (BRS_t, inp["BRS"]), (H_t, inp["H0"]),
            ]
            for l in range(NLAYERS):
                loads += [(WL_t[l][0], inp[f"WL2_ee_{l}"]),
                          (WL_t[l][1], inp[f"WL2_en_{l}"]),
                          (BL_t[l][0], inp[f"BL2_ee_{l}"]),
                          (BL_t[l][1], inp[f"BL2_en_{l}"])]
            for dst, src in loads:
                sync.dma_start(out=dst[:], in_=src[:, :]).then_inc(s_dma0, 16)
                n_dma0 += 1
            sync.dma_start(out=C4_t[:], in_=inp["C4"][0:10, :]).then_inc(s_dma0, 16)
            n_dma0 += 1
            for s in range(NSETS):
                if s >= 1:
                    sync.wait_ge(s_rdma, 16 * s)
                if s >= 2:
                    sync.wait_ge(s_argmm, s - 1)
                src = bass.AP(tensor=inp["R"], offset=s * CELLS,
                              ap=[[NSETS * CELLS, 10], [1, CELLS]])
                sync.dma_start(out=R_t[s % 2][:], in_=src).then_inc(s_rdma, 16)

        @block.tensor
        def _(tensor):
            tensor.wait_ge(s_dma0, 16 * n_dma0)
            for s in range(NSETS):
                tensor.wait_ge(s_rdma, 16 * (s + 1))
                if s >= 1:
                    tensor.wait_ge(s_exp, s)       # ps_arg consumed by exp
                rt = R_t[s % 2]
                for ci, (c0, c1) in enumerate(ARG_CHUNKS):
                    mm = tensor.matmul(ps_arg[:, c0:c1], C4_t[:], rt[:, c0:c1],
                                       start=True, stop=True)
                    if ci == len(ARG_CHUNKS) - 1:
                        mm.then_inc(s_argmm, 1)
                tensor.wait_ge(s_exp, s + 1)       # rbf ready in SBUF
                if s >= 1:
                    tensor.wait_ge(s_tanh, s)      # ps_filt consumed by tanh
                for ci, (c0, c1) in enumerate(ARG_CHUNKS):
                    w = WFe_t if c1 <= 900 else WFn_t
                    mm = tensor.matmul(ps_filt[:, c0:c1], w[:], RBF_t[:, c0:c1],
                                       start=True, stop=True)
                    if ci == len(ARG_CHUNKS) - 1:
                        mm.then_inc(s_filtmm, 1)
            for l in range(NLAYERS):
                for g, sets in enumerate(groups):
                    gidx = l * NG + g
                    if gidx >= 1:
                        tensor.wait_ge(s_tanh2, 2 * gidx)  # ps_z consumed
                    for q, s in enumerate(sets):
                        tensor.wait_ge(s_dvemul, l * NSETS + s + 1)
                        zoff = 512 * (q // 7) + 70 * (q % 7)
                        tensor.matmul(ps_z[:, zoff:zoff + 30], WL_t[l][0][:],
                                      AGG_t[:, q * 70:q * 70 + 30],
                                      start=True, stop=True)
                        mm = tensor.matmul(ps_z[:, zoff + 30:zoff + 70],
                                           WL_t[l][1][:],
                                           AGG_t[:, q * 70 + 30:q * 70 + 70],
                                           start=True, stop=True)
                        if q == len(sets) - 1:
                            mm.then_inc(s_zmm, 1)
            tensor.wait_ge(s_rs, 2)
            tensor.wait_ge(s_tanh2, 2 * NLAYERS * NG)
            tensor.matmul(ps_z[0:2, 0:NSETS], WRe_t[:], RS_e[:],
                          start=True, stop=False)
            tensor.matmul(ps_z[0:2, 0:NSETS], WRn_t[:], RS_n[:],
                          start=False, stop=True).then_inc(s_outmm, 1)

        @block.scalar
        def _(scalar):
            for s in range(NSETS):
                scalar.wait_ge(s_argmm, s + 1)
                if s >= 1:
                    scalar.wait_ge(s_filtmm, s)    # RBF_t consumed
                scalar.activation(RBF_t[:], ps_arg[:], AF.Exp,
                                  bias=CN_t[:, 0:1],
                                  scale=1.0).then_inc(s_exp, 1)
                scalar.wait_ge(s_filtmm, s + 1)
                fb = s * CELLS
                scalar.activation(F_t[:, fb:fb + 900], ps_filt[:, 0:900],
                                  AF.Tanh, bias=BFe_t[:, 0:1], scale=1.0)
                scalar.activation(F_t[:, fb + 900:fb + 1200],
                                  ps_filt[:, 900:1200],
                                  AF.Tanh, bias=BFn_t[:, 0:1],
                                  scale=1.0).then_inc(s_tanh, 1)
            for l in range(NLAYERS):
                for g, sets in enumerate(groups):
                    gidx = l * NG + g
                    scalar.wait_ge(s_zmm, gidx + 1)
                    if gidx >= 1:
                        scalar.wait_ge(s_hadd, gidx)   # T_t consumed
                    nsg = len(sets)
                    nb = min(nsg, 7)
                    nbank = (nsg + 6) // 7
                    scalar.activation(
                        _ap(T_t[:, 0:1], [[490, nbank], [70, nb], [1, 30]]),
                        _ap(ps_z[:, 0:1], [[512, nbank], [70, nb], [1, 30]]),
                        AF.Tanh, bias=BL_t[l][0][:, 0:1],
                        scale=1.0).then_inc(s_tanh2, 1)
                    scalar.activation(
                        _ap(T_t[:, 30:31], [[490, nbank], [70, nb], [1, 40]]),
                        _ap(ps_z[:, 30:31], [[512, nbank], [70, nb], [1, 40]]),
                        AF.Tanh, bias=BL_t[l][1][:, 0:1],
                        scale=1.0).then_inc(s_tanh2, 1)
            scalar.wait_ge(s_outmm, 1)
            scalar.activation(O_t[:], ps_z[0:2, 0:NSETS], AF.Exp,
                              bias=BRS_t[0:2, 0:1],
                              scale=1.0).then_inc(s_actout, 1)

        @block.vector
        def _(vector):
            for s in range(NSETS):
                vector.wait_ge(s_tanh, s + 1)
                vector.memset(_ap(F_t[:, s * CELLS:s * CELLS + 1], [[31, 30]]),
                              0.0).then_inc(s_mask, 1)
            for l in range(NLAYERS):
                for g, sets in enumerate(groups):
                    gidx = l * NG + g
                    for q, s in enumerate(sets):
                        vector.wait_ge(s_mask, s + 1)
                        if l >= 1:
                            vector.wait_ge(s_hadd, (l - 1) * NG + g + 1)
                        if gidx >= 1:
                            vector.wait_ge(s_zmm, gidx)  # AGG consumed
                        hb = s * 70
                        fb = s * CELLS
                        vector.tensor_mul(
                            _ap(P_t[:, 0:1], [[30, 30], [1, 30]]),
                            _ap(H_t[:, hb:hb + 1], [[0, 30], [1, 30]]),
                            _ap(F_t[:, fb:fb + 1], [[30, 30], [1, 30]]))
                        vector.tensor_reduce(
                            AGG_t[:, q * 70:q * 70 + 30],
                            _ap(P_t[:, 0:1], [[30, 30], [1, 30]]),
                            mybir.AxisListType.X, ADD)
                        vector.tensor_mul(
                            _ap(P2_t[:, 0:1], [[10, 30], [1, 10]]),
                            _ap(H_t[:, hb + 60:hb + 61], [[0, 30], [1, 10]]),
                            _ap(F_t[:, fb + 900:fb + 901], [[10, 30], [1, 10]]))
                        vector.tensor_reduce(
                            AGG_t[:, q * 70 + 30:q * 70 + 60],
                            _ap(P2_t[:, 0:1], [[10, 30], [1, 10]]),
                            mybir.AxisListType.X, ADD)
                        vector.tensor_mul(
                            _ap(P3_t[:, 0:1], [[30, 10], [1, 30]]),
                            _ap(H_t[:, hb + 30:hb + 31], [[0, 10], [1, 30]]),
                            _ap(F_t[:, fb + 900:fb + 901], [[1, 10], [10, 30]]))
                        vector.tensor_reduce(
                            AGG_t[:, q * 70 + 60:q * 70 + 70],
                            _ap(P3_t[:, 0:1], [[30, 10], [1, 30]]),
                            mybir.AxisListType.X, ADD).then_inc(s_dvemul, 1)
                    vector.wait_ge(s_tanh2, 2 * (gidx + 1))
                    nsg = len(sets)
                    hslice = H_t[:, sets[0] * 70:(sets[-1] + 1) * 70]
                    vector.tensor_add(hslice, hslice,
                                      T_t[:, 0:nsg * 70]).then_inc(s_hadd, 1)
            vector.wait_ge(s_hadd, NLAYERS * NG)
            vector.tensor_reduce(RS_e[:],
                                 _ap(H_t[:, 0:1], [[70, NSETS], [1, 30]]),
                                 mybir.AxisListType.X, ADD).then_inc(s_rs, 1)
            vector.tensor_reduce(RS_n[:],
                                 _ap(H_t[:, 30:31], [[70, NSETS], [1, 40]]),
                                 mybir.AxisListType.X, ADD).then_inc(s_rs, 1)

        @block.gpsimd
        def _(gpsimd):
            gpsimd.wait_ge(s_actout, 1)
            gpsimd.dma_start(out=y[0:2, :], in_=O_t[:]).then_inc(s_outdma, 16)
            gpsimd.wait_ge(s_outdma, 16)

    return nc


def _host_prep(pos, atoms, emb_ee, wf_ee, bf_ee, wl_ee, bl_ee, wr_ee, br_ee,
               emb_en, wf_en, bf_en, wl_en, bl_en, wr_en, br_en,
               ee_types, en_types):
    f32 = np.float32
    centers = np.linspace(0.0, RBF_CUT, K).astype(f32)

    xyz = pos.astype(np.float64).reshape(NB, NE, 3)
    diff = xyz[:, None, :, :] - xyz[:, :, None, :]        # [nb, j, i, 3]
    d_ee = np.sqrt((diff ** 2).sum(-1)).reshape(NB, CELLS_EE)
    dn = xyz[:, :, None, :] - atoms.astype(np.float64)[None, None, :, :]
    d_en = np.sqrt((dn ** 2).sum(-1)).reshape(NB, CELLS_EN)
    d = np.concatenate([d_ee, d_en], axis=1)
    dsq = d * d
    dsq_hi = dsq.astype(np.float16).astype(np.float64)
    dsq_lo = (dsq - dsq_hi).astype(f32)
    d_hi = d.astype(np.float16).astype(np.float64)
    d_lo = (d - d_hi).astype(f32)
    dsq_hi = dsq_hi.astype(f32)
    d_hi = d_hi.astype(f32)

    def blockdiag(w):
        o = np.zeros((128, 128), f32)
        o[:64, :64] = w
        o[64:, 64:] = w
        return o

    chi = (2.0 * centers).astype(np.float16).astype(f32)
    clo = (2.0 * centers.astype(np.float64) - chi).astype(f32)
    C4 = np.zeros((10, 128), f32)
    for w in range(2):
        C4[5 * w + 0, 64 * w:64 * w + 64] = -1.0
        C4[5 * w + 1, 64 * w:64 * w + 64] = -1.0
        C4[5 * w + 2, 64 * w:64 * w + 64] = chi
        C4[5 * w + 3, 64 * w:64 * w + 64] = clo
        C4[5 * w + 4, 64 * w:64 * w + 64] = chi
    CNEG2 = np.tile(-(centers ** 2), 2).reshape(128, 1).astype(f32)

    def rep2(v):
        return np.tile(np.asarray(v, f32).reshape(-1), 2).reshape(128, 1)

    h0_ee = emb_ee[ee_types].T.astype(f32)
    h0_en = emb_en[en_types].T.astype(f32)
    H0_half = np.concatenate([h0_ee, h0_en], axis=1)
    H0_one = np.concatenate([H0_half, H0_half], axis=0)
    H0 = np.tile(H0_one[:, None, :], (1, NSETS, 1)).reshape(128, NSETS * 70)

    WR2_ee = np.zeros((128, 2), f32)
    WR2_ee[:64, 0] = wr_ee[:, 0]
    WR2_ee[64:, 1] = wr_ee[:, 0]
    WR2_en = np.zeros((128, 2), f32)
    WR2_en[:64, 0] = wr_en[:, 0]
    WR2_en[64:, 1] = wr_en[:, 0]

    const = {
        "C4": C4, "CNEG2": CNEG2,
        "WF2_ee": blockdiag(wf_ee), "WF2_en": blockdiag(wf_en),
        "BF2_ee": rep2(bf_ee), "BF2_en": rep2(bf_en),
        "WR2_ee": WR2_ee.astype(np.float16), "WR2_en": WR2_en.astype(np.float16),
        "H0": np.ascontiguousarray(H0).astype(np.float16),
        "BRS": np.full((128, 1), float(br_ee[0]) + float(br_en[0]), f32),
    }
    for l in range(NLAYERS):
        const[f"WL2_ee_{l}"] = blockdiag(wl_ee[l]).astype(np.float16)
        const[f"WL2_en_{l}"] = blockdiag(wl_en[l]).astype(np.float16)
        const[f"BL2_ee_{l}"] = rep2(bl_ee[l])
        const[f"BL2_en_{l}"] = rep2(bl_en[l])

    in_maps = []
    for c in range(N_CORES):
        lo = c * NW
        R = np.empty((10, NSETS, CELLS), f32)
        for w in range(2):
            R[5 * w + 0] = dsq_hi[lo + w::2][:NSETS]
            R[5 * w + 1] = dsq_lo[lo + w::2][:NSETS]
            R[5 * w + 2] = d_hi[lo + w::2][:NSETS]
            R[5 * w + 3] = d_hi[lo + w::2][:NSETS]
            R[5 * w + 4] = d_lo[lo + w::2][:NSETS]
        m = dict(const)
        m["R"] = np.ascontiguousarray(R.reshape(10, NSETS * CELLS))
        in_maps.append(m)
    return in_maps


def kernel(pos, atoms, emb_ee, wf_ee, bf_ee, wl_ee, bl_ee, wr_ee, br_ee,
           emb_en, wf_en, bf_en, wl_en, bl_en, wr_en, br_en,
           ee_src, ee_dst, ee_types, en_src, en_dst, en_types):
    in_maps = _host_prep(
        np.asarray(pos), np.asarray(atoms), np.asarray(emb_ee),
        np.asarray(wf_ee), np.asarray(bf_ee), np.asarray(wl_ee),
        np.asarray(bl_ee), np.asarray(wr_ee), np.asarray(br_ee),
        np.asarray(emb_en), np.asarray(wf_en), np.asarray(bf_en),
        np.asarray(wl_en), np.asarray(bl_en), np.asarray(wr_en),
        np.asarray(br_en), np.asarray(ee_types), np.asarray(en_types))
    if "nc" not in _CACHE:
        _CACHE["nc"] = _build_module()
    res = run_bass_kernel_spmd(_CACHE["nc"], in_maps, list(range(N_CORES)))
    out = np.concatenate(
        [res.results[c]["y"][0:2, :].T.reshape(NW, 1) for c in range(N_CORES)],
        axis=0)
    return out.astype(np.float32)

